# revision 149
# baseline (speedup 1.0000x reference)
"""Trainium2 Bass kernel for Gemma3 sliding-window attention (v5).

Problem: B=1, T=4096, d_model=2048, 8 query heads / 4 KV heads, head_dim=256,
sliding window 1024, per-head RMSNorm + RoPE (interleaved rotate-half with
cat(freqs,freqs) tables), o_proj.

Sharding (8 cores): 4 KV-head groups x 2 sequence halves. Core (g, s) computes
query heads {2g, 2g+1} and KV head g for query tokens [s*2048, (s+1)*2048).

KV-halo exchange (no recompute): each core projects k/v only for its own 2048
tokens (4 tiles of 512); the two boundary tiles (tokens [1024,2048) of the
core's range, post-RMSNorm+RoPE) are exchanged with the pair partner through
a DRAM-bounce AllGather over replica pairs {(0,1),(2,3),(4,5),(6,7)}. Every
core reads back rank0's contribution as its halo slots: for s=1 cores that is
the true halo; for s=0 cores it is garbage that the exp bias (-1e5) fully
masks, keeping the program SPMD-identical. The collective's ~67us latency
hides behind phases p2-p4 (launched end of p1, first consumed in p5).

fp8 hi+lo DoubleRow matmuls: x and the q/k/v/o weights are split host-side
into e4m3 hi+lo pairs (hi = fp8(v*s), lo = fp8(v*s - hi); x scaled by 16,
weights by 256). A projection contracts (x_hi*W_hi + x_lo*W_hi + x_hi*W_lo)
-- the lo*lo term is below output precision -- as DoubleRow matmuls that
process two K=128 products per instruction at 0.5 cycles/row, i.e. 4x bf16
throughput per product, cutting projection PE time 25% while being MORE
accurate than bf16 operands (~11 effective mantissa bits). The same scheme
runs the o-projection: the attention output is written as a x64-scaled fp8
hi+lo pair and contracted against hi/lo fp8 Wo; the combined x16384 scale is
divided out in the PSUM->SBUF output copies. RMSNorm is scale-invariant, so
the projection scales cancel automatically in the q/k path (v divides its
scale in the PSUM copy).

Softmax denominator: off the PE. The 12 logical P^T j-tiles of a (block,
head) are written into 9 full-width [128,512] buffers (far-edge and
diagonal-edge tiles have complementary column ranges and share buffers); the
denominator is a running bf16 tree-sum of those buffers on the DVE, one
gpsimd partition_all_reduce (broadcast column sums), and a DVE reciprocal.
Scores/y stay bf16: the attention phases are already near the elementwise
engines' latency limit, so fp8 scores do not pay off.

Pipeline (x tiles processed in order [2,3,0,1] so the boundary k/v is ready
early; attention blocks in order [B2,B3,B0,B1]):
  p0: k/q/v(x2)   p1: k/q/v(x3); stage + AllGather   p2: k/q/v(x0)
  p3: k/q/v(x1) + attn(B2); halo readback            p4: attn(B3)+oproj(B2)
  p5: attn(B0)+oproj(B3)    p6: attn(B1)+oproj(B0)   p7: oproj(B1)
Within a projection phase, k and q-head-0 are interleaved by term stage
(q-h0 accumulates in the pp_small PSUM pool so four banks carry both) to
match the chunked DMA arrival order; the norm chain copies its PSUM inputs to
SBUF right after the Squares so projection banks recycle early. q-RoPE runs
on DVE, k-RoPE on Pool, exp on ACT, masks/denominator on DVE, yt fp8
packing on Pool. The o-projection of a block is deferred into the next
block's attention j-loop as PE filler (in-order PE queue), y matmuls lag
their scores by one j-tile to hide exp/mask latency, and output stores
alternate across the two hwdge rings. All loads share the SP ring in
need-order (DMA_ENGINES is one serialized ~330GB/s resource).
"""

import sys

if "/opt/trn_rl_repo" not in sys.path:
    sys.path.insert(0, "/opt/trn_rl_repo")

import numpy as np

try:
    import ml_dtypes
    BF16 = ml_dtypes.bfloat16
    FP8 = ml_dtypes.float8_e4m3fn
except ImportError:
    BF16 = None
    FP8 = None

T, DM, NH, NKV, HD, WIN = 4096, 2048, 8, 4, 256, 1024
EPS, BASE = 1e-6, 10000.0
NG, NS = 4, 2
TL, NQ = 2048, 2048
NKO = 16          # 2048 / 128 contraction subtiles
SCALE = 1.0 / 16.0
NEG = -1.0e5

LOAD_ORDER = [2, 3, 0, 1]   # x-tile processing order
ATT_ORDER = [2, 3, 0, 1]    # attention block order (B2 first, B1 last)
MRELS = [3, 0, 1, 2, 8, 9, 10, 11, 4, 5, 6, 7]
S1, S2 = 16.0, 256.0        # fp8 hi/lo split scales for x and the weights


def _fp8_split(a):
    """a (f32) -> (hi, lo) fp8 e4m3 with a ~= hi + lo."""
    hi = a.astype(FP8)
    lo = (a - hi.astype(np.float32)).astype(FP8)
    return hi, lo

_cache = {}


def _host_prep(x, pos, Wq, Wk, Wv, Wo, q_norm_w, k_norm_w):
    x = np.asarray(x, np.float32).reshape(T, DM)
    xT = np.ascontiguousarray(x.T)
    pos_f = np.asarray(pos).astype(np.float64)
    m = np.arange(128)
    invf = BASE ** (-m / 128.0)

    Wq = np.asarray(Wq, np.float32)
    Wk = np.asarray(Wk, np.float32)
    Wv = np.asarray(Wv, np.float32)
    Wo = np.asarray(Wo, np.float32)
    qnw = np.asarray(q_norm_w, np.float32)
    knw = np.asarray(k_norm_w, np.float32)

    # deinterleave permutation: even logical dims -> subtile 0, odd -> 1
    perm = np.concatenate([2 * np.arange(128), 2 * np.arange(128) + 1])
    epi = (2 * np.arange(128)) % 128
    opi = (2 * np.arange(128) + 1) % 128

    qw2 = np.ascontiguousarray(np.stack([qnw[perm[:128]], qnw[perm[128:]]], axis=1))
    kw2 = np.ascontiguousarray(np.stack([knw[perm[:128]], knw[perm[128:]]], axis=1))

    def permute_heads(W, nheads):
        Wr = W.reshape(nheads, HD, DM)
        return Wr[:, perm, :].reshape(nheads * HD, DM)

    Wqp = permute_heads(Wq, NH)
    Wkp = permute_heads(Wk, NKV)

    ones = np.ones((128, 128), np.float32)

    # masks for 512-wide attention blocks: m=0..3 far edge, m=4..7 diagonal
    jp = np.arange(128)[:, None]
    ip = np.arange(512)[None, :]
    tris = []
    for mm_ in range(4):
        tris.append(jp >= ip + 1 - 128 * mm_)         # far masks F_m
    for mm_ in range(4):
        tris.append(jp <= ip - 128 * mm_)             # diag masks D_{m+8}
    tri = np.concatenate(tris, axis=1).astype(BF16)   # [128, 8*512]

    in_maps = []
    for g in range(NG):
        for s in range(NS):
            t0 = s * 2048
            xT_c = xT[:, t0:t0 + TL]
            p = pos_f[t0:t0 + TL]
            angE = p[None, :] * invf[epi][:, None]
            angO = p[None, :] * invf[opi][:, None]
            tabs = np.stack([np.cos(angE), np.sin(angE),
                             np.cos(angO), np.sin(angO)], axis=0)  # [4,128,TL]

            kbias = np.zeros((128, 24), np.float32)
            if s == 0:
                kbias[:, :8] = NEG
            nconsts = np.concatenate([qw2, kw2, kbias], axis=1)  # [128, 28]

            xh, xl = _fp8_split(np.ascontiguousarray(xT_c) * S1)
            wqh, wql = _fp8_split(np.ascontiguousarray(
                Wqp[2 * g * HD:(2 * g + 2) * HD, :].T) * S2)
            wkh, wkl = _fp8_split(np.ascontiguousarray(
                Wkp[g * HD:(g + 1) * HD, :].T) * S2)
            wvh, wvl = _fp8_split(np.ascontiguousarray(
                Wv[g * HD:(g + 1) * HD, :].T) * S2)
            woh, wol = _fp8_split(np.ascontiguousarray(
                Wo[:, 2 * g * HD:(2 * g + 2) * HD].T) * S2)

            in_maps.append({
                "xh": xh, "xl": xl,
                "tabs": tabs.astype(BF16),
                "nconsts": np.ascontiguousarray(nconsts),
                "wqh": wqh, "wql": wql,
                "wkh": wkh, "wkl": wkl,
                "wvh": wvh, "wvl": wvl,
                "woh": woh, "wol": wol,
                "ones_bf": ones.astype(BF16),
                "kbias": kbias,
                "tri": tri,
            })
    return in_maps


def _build_program():
    if "nc" in _cache:
        return _cache["nc"]

    import concourse.bass as bass
    import concourse.mybir as mybir
    import concourse.tile as tile
    from concourse import bacc, bass_isa
    from contextlib import ExitStack

    f32 = mybir.dt.float32
    bf16 = mybir.dt.bfloat16
    AF = mybir.ActivationFunctionType
    OP = mybir.AluOpType

    nc = bacc.Bacc("TRN2", target_bir_lowering=False, debug=False,
                   enable_asserts=False, num_devices=8)

    fp8 = mybir.dt.float8e4
    PM = mybir.MatmulPerfMode

    xh_d = nc.dram_tensor("xh", [DM, TL], fp8, kind="ExternalInput")
    xl_d = nc.dram_tensor("xl", [DM, TL], fp8, kind="ExternalInput")
    tabs_d = nc.dram_tensor("tabs", [4, 128, TL], bf16, kind="ExternalInput")
    wqh_d = nc.dram_tensor("wqh", [DM, 512], fp8, kind="ExternalInput")
    wql_d = nc.dram_tensor("wql", [DM, 512], fp8, kind="ExternalInput")
    wkh_d = nc.dram_tensor("wkh", [DM, 256], fp8, kind="ExternalInput")
    wkl_d = nc.dram_tensor("wkl", [DM, 256], fp8, kind="ExternalInput")
    wvh_d = nc.dram_tensor("wvh", [DM, 256], fp8, kind="ExternalInput")
    wvl_d = nc.dram_tensor("wvl", [DM, 256], fp8, kind="ExternalInput")
    woh_d = nc.dram_tensor("woh", [512, DM], fp8, kind="ExternalInput")
    wol_d = nc.dram_tensor("wol", [512, DM], fp8, kind="ExternalInput")
    onesbf_d = nc.dram_tensor("ones_bf", [128, 128], bf16, kind="ExternalInput")
    ncst_d = nc.dram_tensor("nconsts", [128, 28], f32, kind="ExternalInput")
    tri_d = nc.dram_tensor("tri", [128, 8 * 512], bf16, kind="ExternalInput")
    o_d = nc.dram_tensor("o_part", [NQ, DM], bf16, kind="ExternalOutput")

    with tile.TileContext(nc) as tc, ExitStack() as ctx:
        cpool = ctx.enter_context(tc.tile_pool(name="consts", bufs=1))
        xpool = ctx.enter_context(tc.tile_pool(name="xt", bufs=2))
        tabpool = ctx.enter_context(tc.tile_pool(name="tab", bufs=2))
        kpool = ctx.enter_context(tc.tile_pool(name="kring", bufs=6))
        vpool = ctx.enter_context(tc.tile_pool(name="vring", bufs=6))
        scpool = ctx.enter_context(tc.tile_pool(name="scratch", bufs=3))
        spool = ctx.enter_context(tc.tile_pool(name="small", bufs=2))
        qpool = ctx.enter_context(tc.tile_pool(name="qt", bufs=8))
        ptpool = ctx.enter_context(tc.tile_pool(name="pt", bufs=9))
        ypool = ctx.enter_context(tc.tile_pool(name="yt", bufs=3))
        opool = ctx.enter_context(tc.tile_pool(name="osb", bufs=4))
        dnpool = ctx.enter_context(tc.tile_pool(name="dn", bufs=1))
        dram = ctx.enter_context(tc.tile_pool(name="dram", bufs=2, space="DRAM"))
        pp_proj = ctx.enter_context(tc.tile_pool(name="pproj", bufs=3, space="PSUM"))
        pp_small = ctx.enter_context(tc.tile_pool(name="psmall", bufs=2, space="PSUM"))
        pp_acc = ctx.enter_context(tc.tile_pool(name="pacc", bufs=3, space="PSUM"))

        # all loads on the SP ring in need-order (DMA_ENGINES is a single
        # serialized resource, so ring choice buys ordering, not bandwidth).
        # wk/x chunks interleaved to match the kp-outer k-projection order;
        # tiny consts slotted in early but behind the first compute chunks.
        wkh_v = wkh_d.ap().rearrange("(ko p) c -> p ko c", p=128)
        wkl_v = wkl_d.ap().rearrange("(ko p) c -> p ko c", p=128)
        wvh_v = wvh_d.ap().rearrange("(ko p) c -> p ko c", p=128)
        wvl_v = wvl_d.ap().rearrange("(ko p) c -> p ko c", p=128)
        wqh_v = wqh_d.ap().rearrange("(ko p) c -> p ko c", p=128)
        wql_v = wql_d.ap().rearrange("(ko p) c -> p ko c", p=128)
        xh_v = xh_d.ap().rearrange("(ko p) t -> p ko t", p=128)  # [128,16,TL]
        xl_v = xl_d.ap().rearrange("(ko p) t -> p ko t", p=128)
        tabs_v = tabs_d.ap().rearrange("f p t -> p f t")

        wkh_sb = cpool.tile([128, NKO, 256], fp8, tag="wkh")
        wkl_sb = cpool.tile([128, NKO, 256], fp8, tag="wkl")
        x20 = 1024  # x tile 2 token offset
        xt2h = xpool.tile([128, NKO, 512], fp8, tag="xh")
        xt2l = xpool.tile([128, NKO, 512], fp8, tag="xl")

        def x_chunk(xt_hl, base, lo, hi):
            if xt_hl[0] is not None:
                nc.sync.dma_start(xt_hl[0][:, lo:hi, :],
                                  xh_v[:, lo:hi, base:base + 512])
            if xt_hl[1] is not None:
                nc.sync.dma_start(xt_hl[1][:, lo:hi, :],
                                  xl_v[:, lo:hi, base:base + 512])

        # stage order is (hi.hi, hi.lo, lo.hi) over (k then q-h0), then q-h1,
        # then v -- load in exactly that demand order
        nc.sync.dma_start(wkh_sb[:, 0:2, :], wkh_v[:, 0:2, :])
        x_chunk((xt2h, None), x20, 0, 2)
        ones_sb = cpool.tile([128, 128], bf16, tag="ones")
        nc.sync.dma_start(ones_sb[:], onesbf_d.ap())
        nc.sync.dma_start(wkh_sb[:, 2:8, :], wkh_v[:, 2:8, :])
        x_chunk((xt2h, None), x20, 2, 8)
        nc.sync.dma_start(wkh_sb[:, 8:16, :], wkh_v[:, 8:16, :])
        x_chunk((xt2h, None), x20, 8, 16)
        wqh_sb = cpool.tile([128, NKO, 512], fp8, tag="wqh")
        wql_sb = cpool.tile([128, NKO, 512], fp8, tag="wql")
        nc.sync.dma_start(wqh_sb[:], wqh_v[:])
        ncst_sb = cpool.tile([128, 28], f32, tag="ncst")
        nc.sync.dma_start(ncst_sb[:], ncst_d.ap())
        pre_tab = tabpool.tile([128, 4, 512], bf16, tag="tab")
        nc.sync.dma_start(pre_tab[:], tabs_v[:, :, x20:x20 + 512])
        qw_sb = ncst_sb[:, 0:2]
        kw_sb = ncst_sb[:, 2:4]
        kb_sb = ncst_sb[:, 4:28]
        x_chunk((None, xt2l), x20, 0, 16)
        nc.sync.dma_start(wkl_sb[:], wkl_v[:])
        nc.sync.dma_start(wql_sb[:], wql_v[:])
        wvh_sb = cpool.tile([128, NKO, 256], fp8, tag="wvh")
        wvl_sb = cpool.tile([128, NKO, 256], fp8, tag="wvl")
        nc.sync.dma_start(wvh_sb[:], wvh_v[:])
        nc.sync.dma_start(wvl_sb[:], wvl_v[:])
        from concourse.hw_specs import get_activation_tables
        _tabnames = list(get_activation_tables(nc.m.arch).keys())
        _setid = _tabnames.index("natural_log_exp_and_others")
        nc.scalar.add_instruction(mybir.InstLoadActFuncSet(
            name=nc.get_next_instruction_name(),
            act_func_set_id=_setid, ins=[], outs=[]))
        eps_sb = cpool.tile([128, 1], f32, tag="eps")
        nc.vector.memset(eps_sb[:], EPS)
        zero_sb = cpool.tile([128, 1], f32, tag="zero")
        nc.vector.memset(zero_sb[:], 0.0)
        # late-needed loads issued behind the startup chunks
        tri_sb = cpool.tile([128, 8 * 512], bf16, tag="tri")
        woh_sb = cpool.tile([128, 4, DM], fp8, tag="woh")
        wol_sb = cpool.tile([128, 4, DM], fp8, tag="wol")

        # collective bounce buffers: free-dim layout
        # [kt_ct4 (2x512) | kt_ct5 | vt_ct4 (4x256) | vt_ct5]
        cc_in = dram.tile([128, 4096], bf16)
        cc_out = dram.tile([2, 128, 4096], bf16)

        kt_tiles = [None] * 6
        vt_tiles = [None] * 6

        OSCALE = 1.0 / (64.0 * S2)

        def emit_oproj_chunk(a, yts, msub, dmh, dq):
            yh_sb, yl_sb = yts
            c0 = (dmh * 2 + dq) * 512
            o_ps = pp_proj.tile([128, 512], f32, tag="pj")
            oterms = [(yh_sb, woh_sb), (yl_sb, woh_sb), (yh_sb, wol_sb)]
            for ti, (yt_, wt_) in enumerate(oterms):
                for pr in range(2):
                    nc.tensor.matmul(
                        o_ps[:],
                        yt_[:, 2 * pr:2 * pr + 2, msub * 128:(msub + 1) * 128],
                        wt_[:, 2 * pr:2 * pr + 2, c0:c0 + 512],
                        start=(ti == 0 and pr == 0), stop=(ti == 2 and pr == 1),
                        perf_mode=PM.DoubleRow)
            o_sb = opool.tile([128, 512], bf16, tag="o")
            ci = msub * 4 + dmh * 2 + dq
            ceng = [None, nc.vector][ci % 2]
            if ceng is None:
                nc.scalar.activation(o_sb[:], o_ps[:], AF.Copy,
                                     bias=0.0, scale=OSCALE)
            else:
                ceng.tensor_scalar(o_sb[:], o_ps[:], OSCALE, None, OP.mult)
            r0_ = a * 512 + msub * 128
            eng = nc.scalar if dmh == 0 else nc.sync
            eng.dma_start(o_d.ap()[r0_:r0_ + 128, c0:c0 + 512], o_sb[:])

        OCHUNKS = [(ms, dm, dq) for ms in range(4) for dm in range(2)
                   for dq in range(2)]

        def norm_pre(src_ps):
            """Square + raw-value copy to SBUF; frees the projection PSUM
            banks right after (the rest of the norm chain reads the copy)."""
            z2 = scpool.tile([128, 2, 512], bf16, tag="z2", bufs=2)
            for u in range(2):
                nc.scalar.activation(z2[:, u, :], src_ps[u][:], AF.Square,
                                     bias=zero_sb[:])
            zc = scpool.tile([128, 2, 512], bf16, tag="zc")
            for u in range(2):
                nc.scalar.copy(zc[:, u, :], src_ps[u][:])
            return z2, zc

        def norm_post(pre, w_sb, tab, dst_h, rope_eng):
            z2, zc = pre
            # sum of squares over the head dim (partitions) via ones-matmul
            ssq = pp_small.tile([128, 512], f32, tag="psm")
            for u in range(2):
                nc.tensor.matmul(ssq[:], ones_sb[:], z2[:, u, :],
                                 start=(u == 0), stop=(u == 1))
            lnt = spool.tile([128, 512], f32, tag="lnt")
            nc.scalar.activation(lnt[:], ssq[:], AF.Ln, bias=eps_sb[:], scale=1.0 / HD)
            rs = spool.tile([128, 512], f32, tag="rs")
            nc.scalar.activation(rs[:], lnt[:], AF.Exp, bias=zero_sb[:], scale=-0.5)
            znw = scpool.tile([128, 2, 512], bf16, tag="znw")
            for u in range(2):
                nc.vector.scalar_tensor_tensor(
                    znw[:, u, :], zc[:, u, :], w_sb[:, u:u + 1], rs[:],
                    OP.mult, OP.mult)
            # rope: out_e = z_e*cosE - z_o*sinE ; out_o = z_o*cosO + z_e*sinO
            t1 = spool.tile([128, 512], bf16, tag="t1")
            rope_eng.tensor_tensor(t1[:], znw[:, 0, :], tab[:, 0, :], OP.mult)
            t2 = spool.tile([128, 512], bf16, tag="t2")
            rope_eng.tensor_tensor(t2[:], znw[:, 1, :], tab[:, 1, :], OP.mult)
            rope_eng.tensor_tensor(dst_h[:, 0, :], t1[:], t2[:], OP.subtract)
            t3 = spool.tile([128, 512], bf16, tag="t3")
            rope_eng.tensor_tensor(t3[:], znw[:, 1, :], tab[:, 2, :], OP.mult)
            t4 = spool.tile([128, 512], bf16, tag="t4")
            rope_eng.tensor_tensor(t4[:], znw[:, 0, :], tab[:, 3, :], OP.mult)
            rope_eng.tensor_tensor(dst_h[:, 1, :], t3[:], t4[:], OP.add)

        NKP = NKO // 2   # DoubleRow processes ko-pairs

        def dr_terms(xth):
            # (weights-hi, x-hi), (weights-hi, x-lo), (weights-lo, x-hi);
            # the lo*lo term is below output precision
            return [(0, xth[0]), (0, xth[1]), (1, xth[0])]

        def proj_kq(xth, tab, slot):
            """k and q-head-0 interleaved by term stage (q-h0 accumulates in
            pp_small so 4 PSUM banks carry both), then q-h1; norms overlap
            the next projection's matmuls."""
            wk_hl = [wkh_sb, wkl_sb]
            terms = dr_terms(xth)
            k_ps = [pp_proj.tile([128, 512], f32, tag="pj", name=f"k{u}")
                    for u in range(2)]
            q0_ps = [pp_small.tile([128, 512], f32, tag="psm", name=f"q0{u}")
                     for u in range(2)]
            for ti, (wi, xt_) in enumerate(terms):
                for kp in range(NKP):
                    for dsub in range(2):
                        nc.tensor.matmul(
                            k_ps[dsub][:],
                            wk_hl[wi][:, 2 * kp:2 * kp + 2,
                                      dsub * 128:(dsub + 1) * 128],
                            xt_[:, 2 * kp:2 * kp + 2, :],
                            start=(ti == 0 and kp == 0),
                            stop=(ti == 2 and kp == NKP - 1),
                            perf_mode=PM.DoubleRow)
                for kp in range(NKP):
                    for u in range(2):
                        nc.tensor.matmul(
                            q0_ps[u][:],
                            [wqh_sb, wql_sb][wi][:, 2 * kp:2 * kp + 2,
                                                 u * 128:(u + 1) * 128],
                            xt_[:, 2 * kp:2 * kp + 2, :],
                            start=(ti == 0 and kp == 0),
                            stop=(ti == 2 and kp == NKP - 1),
                            perf_mode=PM.DoubleRow)
            kpre = norm_pre(k_ps)
            q0pre = norm_pre(q0_ps)
            q1_ps = [pp_proj.tile([128, 512], f32, tag="pj", name=f"q1{u}")
                     for u in range(2)]
            for ti, (wi, xt_) in enumerate(terms):
                for kp in range(NKP):
                    for u in range(2):
                        nc.tensor.matmul(
                            q1_ps[u][:],
                            [wqh_sb, wql_sb][wi][:, 2 * kp:2 * kp + 2,
                                                 256 + u * 128:
                                                 256 + (u + 1) * 128],
                            xt_[:, 2 * kp:2 * kp + 2, :],
                            start=(ti == 0 and kp == 0),
                            stop=(ti == 2 and kp == NKP - 1),
                            perf_mode=PM.DoubleRow)
            kt = kpool.tile([128, 2, 512], bf16, tag="kt")
            norm_post(kpre, kw_sb, tab, kt, nc.gpsimd)
            kt_tiles[slot] = kt
            qt_h0 = qpool.tile([128, 2, 512], bf16, tag="q")
            norm_post(q0pre, qw_sb, tab, qt_h0, nc.vector)
            q1pre = norm_pre(q1_ps)
            return qt_h0, q1pre

        def proj_v(xth, slot, defer_copies=False):
            vt = vpool.tile([128, 4, 256], bf16, tag="vt")
            wv_hl = [wvh_sb, wvl_sb]
            terms = dr_terms(xth)
            v_pss = []
            for vh in range(2):
                v_ps = pp_proj.tile([128, 2, 256], f32, tag="pj")
                v_pss.append(v_ps)
                for ms in range(2):
                    msub = vh * 2 + ms
                    for ti, (wi, xt_) in enumerate(terms):
                        for kp in range(NKP):
                            nc.tensor.matmul(
                                v_ps[:, ms, :],
                                xt_[:, 2 * kp:2 * kp + 2,
                                    msub * 128:(msub + 1) * 128],
                                wv_hl[wi][:, 2 * kp:2 * kp + 2, :],
                                start=(ti == 0 and kp == 0),
                                stop=(ti == 2 and kp == NKP - 1),
                                perf_mode=PM.DoubleRow)

            def do_copies():
                for vh in range(2):
                    for ms in range(2):
                        nc.scalar.activation(vt[:, vh * 2 + ms, :],
                                             v_pss[vh][:, ms, :], AF.Copy,
                                             bias=0.0, scale=1.0 / (S1 * S2))
            vt_tiles[slot] = vt
            if not defer_copies:
                do_copies()
                return None
            return do_copies

        # buffer slot per mrel: (key, ia, ib); edge pairs share a buffer
        def buf_map(mrel):
            if mrel <= 2:
                return ("E%d" % mrel, 0, 128 * (mrel + 1))
            if mrel >= 9:
                return ("E%d" % (mrel - 9), 128 * (mrel - 8), 512)
            return ("M%d" % mrel, 0, 512)

        # denominator tree: after which mi does each add fire, and what gets
        # added (buffer keys; first add initializes acc from two buffers)
        DN_SCHED = {4: ("M3", "M8"), 5: ("E0",), 6: ("E1",), 7: ("E2",),
                    8: ("M4",), 9: ("M5",), 10: ("M6",), 11: ("M7",)}

        att_state = {"yt": None, "a": None}   # previous attended block

        def attn(a, qt_hs, fillers=None):
            # fillers: optional {(h, mi): fn} emitted inside the j-loop
            oc = list(OCHUNKS) if att_state["yt"] is not None else []
            oci = 0
            yh_sb = ypool.tile([128, 4, 512], fp8, tag="yh")
            yl_sb = ypool.tile([128, 4, 512], fp8, tag="yl")
            for h in range(2):
                y0_ps = pp_acc.tile([128, 512], f32, tag="pac")
                y1_ps = pp_acc.tile([128, 512], f32, tag="pac")
                y_ps = [y0_ps, y1_ps]
                bufs = {}
                acc = dnpool.tile([128, 512], bf16, tag="dacc")
                pend = []    # (ia, ib, vtc, jh, pt) awaiting y matmuls
                y_first = True

                def emit_y(ent, last):
                    nonlocal y_first
                    ia_, ib_, vtc_, jh_, pt_ = ent
                    for dh in range(2):
                        nc.tensor.matmul(y_ps[dh][:, ia_:ib_],
                                         vtc_[:, jh_, dh * 128:(dh + 1) * 128],
                                         pt_[:, ia_:ib_], start=y_first,
                                         stop=last, skip_group_check=True)
                    y_first = False

                for mi, mrel in enumerate(MRELS):
                    jt = 4 * a + mrel
                    ct, jh = jt // 4, jt % 4
                    ktc = kt_tiles[ct]
                    vtc = vt_tiles[ct]
                    key, ia, ib = buf_map(mrel)
                    if key not in bufs:
                        bufs[key] = ptpool.tile([128, 512], bf16, tag="p",
                                                name=f"p_{key}")
                    pt = bufs[key]
                    st = pp_small.tile([128, 512], f32, tag="psm")
                    for u in range(2):
                        nc.tensor.matmul(st[:, ia:ib],
                                         ktc[:, u, jh * 128:(jh + 1) * 128],
                                         qt_hs[h][:, u, ia:ib],
                                         start=(u == 0), stop=(u == 1))
                    # y matmuls lag one j-tile so the exp/mask chain of this
                    # tile resolves behind the next tile's scores on the
                    # in-order PE queue
                    if len(pend) == 1:
                        emit_y(pend.pop(0), False)
                    nc.scalar.activation(pt[:, ia:ib], st[:, ia:ib], AF.Exp,
                                         bias=kb_sb[:, jt:jt + 1], scale=SCALE)
                    if mrel < 4:
                        nc.vector.tensor_tensor(
                            pt[:, ia:ib], pt[:, ia:ib],
                            tri_sb[:, mrel * 512 + ia:mrel * 512 + ib], OP.mult)
                    elif mrel >= 8:
                        nc.vector.tensor_tensor(
                            pt[:, ia:ib], pt[:, ia:ib],
                            tri_sb[:, (mrel - 4) * 512 + ia:(mrel - 4) * 512 + ib],
                            OP.mult)
                    pend.append((ia, ib, vtc, jh, pt))
                    # denominator tree on DVE as buffers complete
                    if mi in DN_SCHED:
                        keys = DN_SCHED[mi]
                        if len(keys) == 2:
                            nc.vector.tensor_tensor(
                                acc[:], bufs[keys[0]][:], bufs[keys[1]][:],
                                OP.add)
                        else:
                            nc.vector.tensor_tensor(
                                acc[:], acc[:], bufs[keys[0]][:], OP.add)
                    if fillers and (h, mi) in fillers:
                        fillers[(h, mi)]()
                    if oci < len(oc) and 5 <= mi < 12:
                        ms, dm, dq = oc[oci]
                        emit_oproj_chunk(att_state["a"], att_state["yt"],
                                         ms, dm, dq)
                        oci += 1
                emit_y(pend.pop(0), True)
                dn = spool.tile([128, 512], f32, tag="dnb", bufs=1)
                nc.gpsimd.partition_all_reduce(
                    dn[:], acc[:], channels=128, reduce_op=bass_isa.ReduceOp.add)
                rc = spool.tile([128, 512], f32, tag="rc")
                nc.vector.reciprocal_approx_fast(rc[:], dn[:])
                for dh in range(2):
                    # yt as fp8 hi+lo pair (scaled x64) for the DoubleRow
                    # o-projection
                    yf = spool.tile([128, 512], bf16, tag="yf")
                    nc.vector.scalar_tensor_tensor(
                        yf[:], y_ps[dh][:], 64.0, rc[:], OP.mult, OP.mult)
                    nc.gpsimd.tensor_copy(yh_sb[:, 2 * h + dh, :], yf[:])
                    nc.gpsimd.tensor_tensor(yl_sb[:, 2 * h + dh, :], yf[:],
                                            yh_sb[:, 2 * h + dh, :],
                                            OP.subtract)
            while oci < len(oc):
                ms, dm, dq = oc[oci]
                emit_oproj_chunk(att_state["a"], att_state["yt"], ms, dm, dq)
                oci += 1
            att_state["yt"], att_state["a"] = (yh_sb, yl_sb), a

        # ================= phases =================
        qts = {}
        cur_x, cur_tab = (xt2h, xt2l), pre_tab
        x2_saved = (xt2h, xt2l)
        for p in range(4):
            xi = LOAD_ORDER[p]
            xth, tab = cur_x, cur_tab
            if p == 1:
                # v(x2) deferred out of the DMA-bound p0: runs here on
                # resident data while x3's tail still arrives. Must be
                # emitted before the x0 prefetch reuses x2's buffers.
                proj_v(x2_saved, 4)
                nc.sync.dma_start(
                    cc_in[:, 2048:3072],
                    vt_tiles[4][:].rearrange("p m d -> p (m d)"))
            # prefetch next x tile + tab (SP ring)
            if p + 1 < 4:
                t1_ = LOAD_ORDER[p + 1] * 512
                xth_n = xpool.tile([128, NKO, 512], fp8, tag="xh")
                nc.sync.dma_start(xth_n[:], xh_v[:, :, t1_:t1_ + 512])
                xtl_n = xpool.tile([128, NKO, 512], fp8, tag="xl")
                nc.sync.dma_start(xtl_n[:], xl_v[:, :, t1_:t1_ + 512])
                ntab = tabpool.tile([128, 4, 512], bf16, tag="tab")
                nc.sync.dma_start(ntab[:], tabs_v[:, :, t1_:t1_ + 512])
                cur_x, cur_tab = (xth_n, xtl_n), ntab
            if p == 0:
                nc.sync.dma_start(tri_sb[:], tri_d.ap())

            qt_h0, q1pre = proj_kq(xth, tab, 2 + xi)
            if p != 0:
                vcopies = proj_v(xth, 2 + xi, defer_copies=False)
            qt_h1 = qpool.tile([128, 2, 512], bf16, tag="q")
            norm_post(q1pre, qw_sb, tab, qt_h1, nc.vector)
            qts[xi] = [qt_h0, qt_h1]

            if p == 0:
                # stage tile ct4's k as soon as its norm/rope lands (its v is
                # projected and staged at the start of p1)
                nc.sync.dma_start(
                    cc_in[:, 0:1024],
                    kt_tiles[4][:].rearrange("p u t -> p (u t)"))

            if p == 1:
                # stage boundary k/v (post-norm/rope) and launch the pair
                # AllGather; readback is issued in p3 (SP ring) so the
                # CC-semaphore wait never blocks time-critical DMAs.
                nc.sync.dma_start(
                    cc_in[:, 1024:2048],
                    kt_tiles[5][:].rearrange("p u t -> p (u t)"))
                nc.sync.dma_start(
                    cc_in[:, 3072:4096],
                    vt_tiles[5][:].rearrange("p m d -> p (m d)"))
                nc.gpsimd.collective_compute(
                    "AllGather",
                    mybir.AluOpType.bypass,
                    replica_groups=[[0, 1], [2, 3], [4, 5], [6, 7]],
                    ins=[cc_in.opt()],
                    outs=[cc_out.opt()],
                )
                nc.sync.dma_start(
                    woh_sb[:], woh_d.ap().rearrange("(hd p) c -> p hd c", p=128))
                nc.sync.dma_start(
                    wol_sb[:], wol_d.ap().rearrange("(hd p) c -> p hd c", p=128))
            if p == 3:
                # halo readback: rank0 of the pair (true halo on s=1 cores;
                # bias-masked garbage on s=0 cores)
                kt0 = kpool.tile([128, 2, 512], bf16, tag="kt")
                nc.sync.dma_start(kt0[:].rearrange("p u t -> p (u t)"),
                                  cc_out[0][:, 0:1024])
                kt1 = kpool.tile([128, 2, 512], bf16, tag="kt")
                nc.sync.dma_start(kt1[:].rearrange("p u t -> p (u t)"),
                                  cc_out[0][:, 1024:2048])
                vt0 = vpool.tile([128, 4, 256], bf16, tag="vt")
                nc.sync.dma_start(vt0[:].rearrange("p m d -> p (m d)"),
                                  cc_out[0][:, 2048:3072])
                vt1 = vpool.tile([128, 4, 256], bf16, tag="vt")
                nc.sync.dma_start(vt1[:].rearrange("p m d -> p (m d)"),
                                  cc_out[0][:, 3072:4096])
                kt_tiles[0], kt_tiles[1] = kt0, kt1
                vt_tiles[0], vt_tiles[1] = vt0, vt1
                attn(ATT_ORDER[0], qts[ATT_ORDER[0]],
                     fillers={(0, 2): vcopies} if vcopies else None)

        for p in range(4, 7):
            attn(ATT_ORDER[p - 3], qts[ATT_ORDER[p - 3]])
        # final o-projection (last attended block)
        for (ms, dm, dq) in OCHUNKS:
            emit_oproj_chunk(att_state["a"], att_state["yt"], ms, dm, dq)

    nc.compile()
    _cache["nc"] = nc
    return nc


def _run(inputs, trace=False):
    from concourse.bass_utils import run_bass_kernel_spmd

    nc = _build_program()
    in_maps = _host_prep(**inputs)
    res = run_bass_kernel_spmd(nc, in_maps, core_ids=list(range(8)), trace=trace)
    full = np.zeros((T, DM), np.float32)
    for g in range(NG):
        for s in range(NS):
            full[s * 2048:(s + 1) * 2048] += np.asarray(
                res.results[g * 2 + s]["o_part"], dtype=np.float32)
    return full.reshape(1, T, DM), res


def kernel(**inputs):
    return _run(inputs, trace=False)[0]
